# revision 104
# baseline (speedup 1.0000x reference)
"""Trainium2 Bass kernel: fused multi-head attention (N=4, L=2048, E=2048, H=16).

Sharding (8 cores): data-parallel over the 4 batches x tensor-parallel over 2
head-groups of 8 heads.  Core c handles batch c//2, head-group c%2.  Each core
computes Q/K/V projections for its head group, masked softmax attention, and
the partial output projection against its row-slice of Wo.  The two partials
per batch are summed on the host (the Wo row-parallel all-reduce) and the
output bias is added there too.

Precision plan (rel-err budget 2e-2):
  - Q/K projections and QK^T scores run as fp8-e4m3 DoubleRow matmuls (0.5
    cycles/row = 2x bf16).  The stationary operand is error-compensated as an
    (hi, lo) e4m3 pair summed by the two DoubleRow groups, so only the moving
    operand's quantization error survives (~0.7% per path; three paths).
  - V path / AV / output projection stay 16-bit (their quantization errors
    do not average out over the contraction).  pt and V use fp16 (10-bit
    mantissa, and 2-byte dtype keeps DVE in 2x mode for the denominator tree).

Per-core layout (all matmuls contraction-on-partitions, no on-device
transposes -- the host ships pre-transposed activations/weights):
  - K^T per head: [d=128, LK] fp8 hi/lo pair via DoubleRow lhsT=(Wk_hi,Wk_lo)
  - scores S^T tile: [kblock=128, LH] = DoubleRow((K^T_hi,K^T_lo) pair, q)
  - P = exp(S^T * E^-0.5 + mask_bias[k]) on ScalarE, output fp16
  - out^T_h accumulated in PSUM via lhsT=V chunk (fp16), rhs=P chunk
  - softmax denominator: DVE pairwise fp16 tree over P chunks, a ones-matmul
    partition reduce+broadcast, then DVE reciprocal + multiply
  - output projection: fp16, lhsT=A^T blocks, rhs=Wo^T, accumulated over heads

Scheduling: head-level software pipeline -- head j's ScalarE-bound chunk loop
interleaves head j+1's Q-projection matmuls (4 per chunk) plus the deferred
softmax finalize of head j-1, scores run one chunk ahead of exp, and all
DMAs are merged into few large descriptors (the shared HWDGE descriptor
generator costs ~630ns per DMA instruction regardless of size).
"""

from contextlib import ExitStack

import numpy as np
import ml_dtypes

P = 128          # SBUF partitions
D = 128          # head dim
G = 2            # head groups (tensor-parallel degree per batch)
NCORES = 8
BF16 = ml_dtypes.bfloat16
F16 = np.float16
E4M3 = ml_dtypes.float8_e4m3fn
MASK_BIAS = -60.0

_BUILT = {}
LAST_EXEC_NS = None
PHASE_MARKS = {}


def _build(L, E, HL, LK=None, reps=1):
    """Build the per-core Bass module (same program on every core).

    LK is the (padded) compacted key length: the host drops masked-out keys
    -- they contribute exactly zero to both the attention numerator and
    denominator -- and pads to a multiple of 128.  reps>1 repeats the whole
    computation serially inside one NEFF (scratch WAW deps order the reps)
    -- used only for slope-based HW timing."""
    if LK is None:
        LK = L
    import concourse.bass as bass
    import concourse.tile as tile
    from concourse import mybir

    bf16 = mybir.dt.bfloat16
    f16 = mybir.dt.float16
    f8 = mybir.dt.float8e4
    f32 = mybir.dt.float32
    DR = mybir.MatmulPerfMode.DoubleRow

    EH = HL * D          # local projection width
    IC = E // P          # contraction chunks (projections)
    ICH = IC // 2        # half of the contraction chunks (split W/X loads)
    KC = LK // P         # key chunks (attention)
    ET = E // 512        # 512-wide e tiles (out proj)
    VW = min(512, EH // 2)  # v-proj dh tile width (within one W half-load)
    VT = EH // VW
    LH = min(1024, L)    # attention l-half width
    NLH = L // LH
    NTH = LH // 512
    SCALE = float(E) ** -0.5

    nc = bass.Bass(num_swdge_queues=4)
    xqt = nc.dram_tensor("xqt", [E, L], f8, kind="ExternalInput")
    xkt = nc.dram_tensor("xkt", [E, LK], f8, kind="ExternalInput")
    xvt = nc.dram_tensor("xvt", [E, LK], f16, kind="ExternalInput")
    wqt = nc.dram_tensor("wqt", [E, 2, EH], f8, kind="ExternalInput")
    # K weights ship pre-split in dh-halves so each half loads as one 3-dim
    # DMA (the pair axis flattens contiguously)
    wkt0 = nc.dram_tensor("wkt0", [E, 2, EH // 2], f8, kind="ExternalInput")
    wkt1 = nc.dram_tensor("wkt1", [E, 2, EH // 2], f8, kind="ExternalInput")
    wvt = nc.dram_tensor("wvt", [E, EH], f16, kind="ExternalInput")
    wot = nc.dram_tensor("wot", [EH, E], f16, kind="ExternalInput")
    mbias = nc.dram_tensor("mbias", [P, KC], f32, kind="ExternalInput")
    out = nc.dram_tensor("out", [L, E], f16, kind="ExternalOutput")

    kt_d = nc.dram_tensor("kt_scratch", [HL, P, 2, LK], f8)
    v_d = nc.dram_tensor("v_scratch", [LK, EH], f16)

    def dup2(ap):
        """View a [p, n] AP as [p, 2, n] with a stride-0 pair axis (the
        DoubleRow moving operand when both groups share one tensor)."""
        return ap.unsqueeze(1).broadcast_to([ap.shape[0], 2, ap.shape[1]])

    with tile.TileContext(nc) as tc, ExitStack() as ctx:
        # All SBUF pools are opened for the whole kernel so no SBUF address is
        # ever reused across pools (cross-pool aliasing generates WAR waits
        # against many DMA-queue processors -> "too many sync wait commands").
        singles = ctx.enter_context(tc.tile_pool(name="singles", bufs=1))
        at_pool = ctx.enter_context(tc.tile_pool(name="at", bufs=1))
        xp = ctx.enter_context(tc.tile_pool(name="xp", bufs=2))
        wp = ctx.enter_context(tc.tile_pool(name="wp", bufs=3))
        op_ = ctx.enter_context(tc.tile_pool(name="op", bufs=4))
        kqp = ctx.enter_context(tc.tile_pool(name="kq", bufs=2))
        qtp = ctx.enter_context(tc.tile_pool(name="qt", bufs=3))
        vp = ctx.enter_context(tc.tile_pool(name="vpool", bufs=2))
        ptp = ctx.enter_context(tc.tile_pool(name="ptp", bufs=5))
        trp = ctx.enter_context(tc.tile_pool(name="trp", bufs=2))
        dnp = ctx.enter_context(tc.tile_pool(name="dnp", bufs=2))

        mb_t = singles.tile([P, KC], f32)
        ones16 = singles.tile([P, P], f16)
        nc.vector.memset(ones16, 1.0)

        at_t = at_pool.tile([P, HL, L], f16)

        # ---- K / V projections (spilled to DRAM scratch) ----
        EHH = EH // 2  # W loaded in two dh-halves so loads pipeline



        def proj_k(rep, pp):
            """fp8 DoubleRow K-projection; spills per-head (hi, lo) e4m3
            K^T pairs to kt_d.  One DMA per W half / X tile / spill pair --
            the shared HWDGE descriptor generator costs ~630ns per DMA
            instruction regardless of size."""
            xv = xkt.rearrange("(c p) l -> p c l", p=P)
            wvs = [wkt0.rearrange("(c p) two m -> p c (two m)", p=P),
                   wkt1.rearrange("(c p) two m -> p c (two m)", p=P)]
            if True:
                wts = [wp.tile([P, IC, 2, EHH], f8, tag="w", name=f"wkt{i}")
                       for i in range(2)]

                tiles = []
                off = 0
                while off < LK:
                    w = min(512, LK - off)
                    tiles.append((off, w))
                    off += w

                # first W half + X tile stream in c-quarters so the first
                # accumulation chain starts ~8us earlier; remaining X tiles
                # load next (needed by h=0 already in h-outer order), W half
                # 1 last (first needed at h=4)
                xts = [xp.tile([P, IC, tw], f8, tag=f"xk{tw}",
                               name=f"xk{ti}",
                               bufs=sum(1 for _, w2 in tiles if w2 == tw))
                       for ti, (_, tw) in enumerate(tiles)]
                w0_flat = wts[0].rearrange("p c two m -> p c (two m)")
                QC = IC // 8
                for q in range(8):
                    if q >= 2 and q % 2 == 1:
                        continue  # eighths only for the first chunk pair
                    n = 1 if q < 2 else 2
                    cs = slice(q * QC, (q + n) * QC)
                    nc.sync.dma_start(out=w0_flat[:, cs], in_=wvs[0][:, cs])
                    nc.sync.dma_start(
                        out=xts[0][:, cs, :tiles[0][1]],
                        in_=xv[:, cs, tiles[0][0]:tiles[0][0] + tiles[0][1]])
                for ti, (off, w) in enumerate(tiles[1:], 1):
                    nc.sync.dma_start(
                        out=xts[ti][:, :, :w], in_=xv[:, :, off:off + w])
                nc.sync.dma_start(
                    out=wts[1].rearrange("p c two m -> p c (two m)"),
                    in_=wvs[1])

                def w_slice(j0, j1):  # dh range -> (tile, local slice)
                    wh = j0 // EHH
                    assert (j1 - 1) // EHH == wh
                    return wts[wh][:, :, :, j0 - wh * EHH:j1 - wh * EHH]

                for h in range(HL):
                    wsl = w_slice(h * D, (h + 1) * D)
                    for ti, (off, w) in enumerate(tiles):
                        xt = xts[ti]
                        ps = pp.tile([P, 512], f32, tag="ps")
                        for c in range(IC):
                            nc.tensor.matmul(
                                ps[:, :w], lhsT=wsl[:, c],
                                rhs=dup2(xt[:, c, :w]),
                                start=(c == 0), stop=(c == IC - 1),
                                perf_mode=DR)
                        sp = op_.tile([P, 2, 512], f8, tag="o8", name="ksp",
                                      bufs=10)
                        nc.scalar.copy(out=sp[:, 0, :w], in_=ps[:, :w])
                        nc.vector.tensor_sub(
                            out=sp[:, 1, :w], in0=ps[:, :w], in1=sp[:, 0, :w])
                        nc.scalar.dma_start(
                            out=kt_d[h, :, :, off:off + w], in_=sp[:, :, :w])

        def proj_v(rep, pp):
            """bf16 V-projection; spills natural-layout fp16 V to v_d."""
            xv = xvt.rearrange("(c p) l -> p c l", p=P)
            wv = wvt.rearrange("(c p) m -> p c m", p=P)
            if True:
                wts = [wp.tile([P, IC, EHH], f16, tag="w", name=f"wvt{i}")
                       for i in range(2)]
                nc.sync.dma_start(out=wts[0], in_=wv[:, :, 0:EHH])
                nc.sync.dma_start(out=wts[1], in_=wv[:, :, EHH:EH])

                def w_slice(j0, j1):
                    wh = j0 // EHH
                    assert (j1 - 1) // EHH == wh
                    return wts[wh][:, :, j0 - wh * EHH:j1 - wh * EHH]

                tiles = []
                off = 0
                while off < LK:
                    w = min(512, LK - off)
                    tiles.append((off, w))
                    off += w
                for ti, (off, w) in enumerate(tiles):
                    xt = xp.tile([P, IC, 512], f16, tag="xv")
                    nc.sync.dma_start(
                        out=xt[:, :, :w], in_=xv[:, :, off:off + w])
                    # vt-outer: the wts[1] half isn't touched until half a
                    # tile in, hiding its DMA behind the vt=0 matmuls
                    for vt_ in range(VT):
                        for kb in range(w // P):
                            ps = pp.tile([P, VW], f32, tag="ps")
                            wsl = w_slice(vt_ * VW, (vt_ + 1) * VW)
                            for c in range(IC):
                                nc.tensor.matmul(
                                    ps, lhsT=xt[:, c, kb * P:(kb + 1) * P],
                                    rhs=wsl[:, c],
                                    start=(c == 0), stop=(c == IC - 1))
                            ot = op_.tile([P, VW], f16, tag="o16", bufs=4)
                            nc.vector.tensor_copy(out=ot, in_=ps)
                            r0 = off + kb * P
                            nc.scalar.dma_start(
                                out=v_d[r0:r0 + P,
                                        vt_ * VW:(vt_ + 1) * VW],
                                in_=ot)

        # ---- fused Q-projection + attention ----
        # Head-level software pipeline: head j's ScalarE-bound chunk loop has
        # ~4 PE-idle slots per chunk, so head j+1's Q-projection matmuls are
        # interleaved into it (4 per chunk).  The qt cast lands mid-loop and
        # the per-head finalize (tree root, PSUM->SBUF stage, partition
        # reduce, divide) trails one head behind, keeping every engine busy
        # at the exp cadence.
        xq_v = xqt.rearrange("(c p) l -> p c l", p=P)
        wq_v = wqt.rearrange("(c p) two m -> p c two m", p=P)
        v_view = v_d.rearrange("(c p) m -> p c m", p=P)

        def run_attention(rep, otp, qpp):
          nc.sync.dma_start(out=mb_t, in_=mbias[:, :])
          wq_flat = wqt.rearrange("(c p) two m -> p c (two m)", p=P)
          wq_halves = []
          for wh in range(2):  # ic-halves
              wqh = wp.tile([P, ICH, 2, EH], f8, tag="w", name=f"wqt{wh}")
              nc.sync.dma_start(
                  out=wqh.rearrange("p c two m -> p c (two m)"),
                  in_=wq_flat[:, wh * ICH:(wh + 1) * ICH])
              wq_halves.append(wqh)

          jobs = [(lh, h) for lh in range(NLH) for h in range(HL)]
          xq_tiles = {}
          ktv = {}
          q_ps_t = {}
          qt_t_t = {}
          pending_fin = [None]
          # Qproj matmuls interleaved per chunk, front-loaded so the qt cast
          # can land one chunk earlier
          QSCHED = [5, 5, 5, 5, 4, 4, 4, 0, 0]
          q_cursor = {}

          def load_xq(lh):
              if lh in xq_tiles or lh >= NLH:
                  return
              halves = []
              for wh in range(2):
                  xqh = xp.tile([P, ICH, LH], f8, tag="xq", name=f"xq{wh}")
                  nc.sync.dma_start(
                      out=xqh,
                      in_=xq_v[:, wh * ICH:(wh + 1) * ICH,
                               lh * LH:(lh + 1) * LH])
                  halves.append(xqh)
              xq_tiles[lh] = halves

          v_pairs = {}

          def prefetch_ktv(j):
              if j in ktv or j >= len(jobs):
                  return
              lh, h = jobs[j]
              kt_t = kqp.tile([P, 2, LK], f8, tag="kt")
              nc.sync.dma_start(out=kt_t, in_=kt_d[h])
              # V loaded per head PAIR: 512-byte runs avoid the sub-512B
              # descriptor penalty and halve the DMA count
              hp = h // 2
              if (lh, hp) not in v_pairs:
                  vt2 = vp.tile([P, KC, 2 * D], f16, tag="v")
                  nc.sync.dma_start(
                      out=vt2, in_=v_view[:, :, hp * 2 * D:(hp + 1) * 2 * D])
                  v_pairs[(lh, hp)] = vt2
              v_t = v_pairs[(lh, hp)][:, :, (h % 2) * D:(h % 2 + 1) * D]
              ktv[j] = (kt_t, v_t)

          def emit_qproj_part(j, n):
              if j >= len(jobs) or n == 0:
                  return
              lh, h = jobs[j]
              if j not in q_ps_t:
                  q_ps_t[j] = qpp.tile([P, LH], f32, tag="q", name="q_ps")
              q_ps = q_ps_t[j]
              base = q_cursor.get(j, 0)
              q_cursor[j] = base + n
              for k in range(n):
                  nt, c = divmod(base + k, IC)
                  nc.tensor.matmul(
                      q_ps[:, nt * 512:(nt + 1) * 512],
                      lhsT=wq_halves[c // ICH][
                          :, c % ICH, :, h * D:(h + 1) * D],
                      rhs=dup2(xq_tiles[lh][c // ICH][
                          :, c % ICH, nt * 512:(nt + 1) * 512]),
                      start=(c == 0), stop=(c == IC - 1), perf_mode=DR)

          def emit_qt(j):
              if j >= len(jobs):
                  return
              q_ps = q_ps_t.pop(j)
              qt_t = qtp.tile([P, LH], f8, tag="qt")
              for nt in range(NTH):
                  nc.vector.tensor_copy(
                      out=qt_t[:, nt * 512:(nt + 1) * 512],
                      in_=q_ps[:, nt * 512:(nt + 1) * 512])
              qt_t_t[j] = qt_t

          def emit_red():
              # partition reduce+broadcast via ones-matmul (fp16 moving
              # operand: 427ns on PE)
              p = pending_fin[0]
              if p is None or p.get("red") is not None:
                  return
              red = stp.tile([P, LH], f32, tag="st", name="red")
              for nt in range(NTH):
                  nc.tensor.matmul(red[:, nt * 512:(nt + 1) * 512],
                                   lhsT=ones16,
                                   rhs=p["den"][:, nt * 512:(nt + 1) * 512],
                                   start=True, stop=True)
              p["red"] = red

          def emit_fin_dve():
              p = pending_fin[0]
              if p is None:
                  return
              if p.get("red") is None:
                  emit_red()
              pending_fin[0] = None
              rec = dnp.tile([P, LH], f16, tag="den", bufs=1)
              with nc.allow_low_precision(
                      reason="fp16 softmax denominator: |den|<2k, 0.05% rel"):
                  nc.vector.reciprocal(out=rec, in_=p["red"])
              nc.vector.tensor_mul(
                  out=at_t[:, p["h"], p["l0"]:p["l0"] + LH],
                  in0=p["tmp"], in1=rec)

          with tc.tile_pool(name=f"stps{rep}", bufs=2, space="PSUM") as stp, \
               tc.tile_pool(name=f"tmp{rep}", bufs=2) as tmpp:
            # prologue: first head's inputs + Q-projection stand alone
            load_xq(0)
            prefetch_ktv(0)
            emit_qproj_part(0, NTH * IC)
            emit_qt(0)

            for j, (lh, h) in enumerate(jobs):
                l0 = lh * LH
                kt_t, v_t = ktv.pop(j)
                qt_t = qt_t_t.pop(j)
                ot_ps = otp.tile([P, LH], f32, tag="ot")
                # binary-counter pairwise fp16 tree for the denominator
                # (2-byte operands keep DVE in 2x mode)
                stack = []  # (rank, tile)

                def push(t):
                    r = 0
                    while stack and stack[-1][0] == r:
                        _, prev = stack.pop()
                        s = trp.tile([P, LH], f16, tag=f"tr{r}", name=f"tr{r}",
                                     bufs=2 if r < 2 else 1)
                        nc.vector.tensor_add(out=s, in0=prev, in1=t)
                        t = s
                        r += 1
                    stack.append((r, t))

                def emit_scores(c):
                    st = stp.tile([P, LH], f32, tag="st", name="st")
                    for nt in range(NTH):
                        nc.tensor.matmul(
                            st[:, nt * 512:(nt + 1) * 512],
                            lhsT=kt_t[:, :, c * P:(c + 1) * P],
                            rhs=dup2(qt_t[:, nt * 512:(nt + 1) * 512]),
                            start=True, stop=True, perf_mode=DR)
                    return st

                # scores run one chunk ahead so exp latency never blocks PE
                st_next = emit_scores(0)
                for c in range(KC):
                    if c == 0:
                        prefetch_ktv(j + 1)
                        if h == HL - 2:
                            load_xq(lh + 1)
                    st = st_next
                    pt = ptp.tile([P, LH], f16, tag="pt")
                    nc.scalar.activation(
                        out=pt, in_=st,
                        func=mybir.ActivationFunctionType.Exp,
                        bias=mb_t[:, c:c + 1], scale=SCALE)
                    if c + 1 < KC:
                        st_next = emit_scores(c + 1)
                    push(pt)
                    for nt in range(NTH):
                        nc.tensor.matmul(
                            ot_ps[:, nt * 512:(nt + 1) * 512],
                            lhsT=v_t[:, c],
                            rhs=pt[:, nt * 512:(nt + 1) * 512],
                            start=(c == 0), stop=(c == KC - 1))
                    emit_qproj_part(j + 1, QSCHED[c])
                    if q_cursor.get(j + 1, 0) == NTH * IC \
                            and (j + 1) not in qt_t_t and j + 1 < len(jobs):
                        emit_qt(j + 1)
                    if c == 1:
                        emit_fin_dve()
                # stage ot out of PSUM first (frees the single ot bank for
                # the next head's AV c0); the tree root can trail
                tmp = tmpp.tile([P, LH], f16, tag="tmp")
                for nt in range(NTH):
                    nc.vector.tensor_copy(
                        out=tmp[:, nt * 512:(nt + 1) * 512],
                        in_=ot_ps[:, nt * 512:(nt + 1) * 512])
                while len(stack) > 1:
                    _, b = stack.pop()
                    r, a = stack.pop()
                    s = trp.tile([P, LH], f16, tag=f"trf{r}", name="trf",
                                 bufs=1)
                    nc.vector.tensor_add(out=s, in0=a, in1=b)
                    stack.append((r + 1, s))
                pending_fin[0] = {"den": stack.pop()[1], "tmp": tmp,
                                  "h": h, "l0": l0}
            emit_fin_dve()

        # ---- output projection ----
        wot_v = wot.rearrange("(h p) e -> p h e", p=P)
        HLH = max(1, HL // 2)

        def run_outproj(rep):
          # Wo^T loads into the weight pool slots freed after the Q weights.
          wo_halves = []
          for wh in range(2 if HL > 1 else 1):
              woh = wp.tile([P, HLH, E], f16, tag="w", name=f"wot{wh}")
              nc.sync.dma_start(
                  out=woh, in_=wot_v[:, wh * HLH:(wh + 1) * HLH])
              wo_halves.append(woh)

          with tc.tile_pool(name=f"oo{rep}", bufs=2) as oop, \
               tc.tile_pool(name=f"ops{rep}", bufs=4, space="PSUM") as opp:
            for lb in range(L // P):
                for eh in range(ET // 2):
                    ot = oop.tile([P, 2, 512], f16, tag="o", bufs=3)
                    for e2 in range(2):
                        et = eh * 2 + e2
                        ps = opp.tile([P, 512], f32, tag="ps")
                        for h in range(HL):
                            nc.tensor.matmul(
                                ps, lhsT=at_t[:, h, lb * P:(lb + 1) * P],
                                rhs=wo_halves[h // HLH][
                                    :, h % HLH, et * 512:(et + 1) * 512],
                                start=(h == 0), stop=(h == HL - 1))
                        nc.vector.tensor_copy(out=ot[:, e2], in_=ps)
                    nc.sync.dma_start(
                        out=out[lb * P:(lb + 1) * P,
                                eh * 1024:(eh + 1) * 1024],
                        in_=ot)

        for rep in range(reps):
            PHASE_MARKS.clear()
            # attention's ot/q accumulator pools open BESIDE the projection
            # pool (4+4 banks): they land on disjoint PSUM banks, so the
            # attention prologue's Q-projection has no WAR wait on V's tail
            with tc.tile_pool(name=f"otps{rep}", bufs=1,
                              space="PSUM") as otp, \
                 tc.tile_pool(name=f"qps{rep}", bufs=1,
                              space="PSUM") as qpp:
                # one PSUM pool spans both projections: K->V ring reuse
                # carries per-slot WAR deps instead of a phase barrier
                with tc.tile_pool(name=f"ps_kv{rep}", bufs=4,
                                  space="PSUM") as kvpp:
                    PHASE_MARKS["k"] = nc.next_id()
                    proj_k(rep, kvpp)
                    PHASE_MARKS["v"] = nc.next_id()
                    proj_v(rep, kvpp)
                PHASE_MARKS["attn"] = nc.next_id()
                run_attention(rep, otp, qpp)
            PHASE_MARKS["outproj"] = nc.next_id()
            run_outproj(rep)
            PHASE_MARKS["end"] = nc.next_id()

    # Split multi-wait sync_infos (TRN2 instructions carry at most one wait;
    # only the Bacc path runs this pass by default).
    import bass_rust
    bass_rust.move_matmul_waits_to_ldweights(nc.m)
    bass_rust.generate_event_semaphores(nc)
    return nc


def _get_nc(L, E, HL, LK=None):
    key = (L, E, HL, LK)
    if key not in _BUILT:
        _BUILT[key] = _build(L, E, HL, LK=LK)
    return _BUILT[key]


def _hilo(w):
    """e4m3 error-compensated pair along a new axis 1: w ~= hi + lo."""
    hi = w.astype(E4M3)
    lo = (w - hi.astype(np.float32)).astype(E4M3)
    return np.ascontiguousarray(np.stack([hi, lo], axis=1))


def _core_inputs(query_n, kc_n, vc_n, mb_n, Wq, Wk, Wv, Wo, g, HL, LK):
    """Host-side shard prep for one core: transpose + cast the batch's
    (key-compacted) activations and the head-group's weight slices."""
    EH = HL * D
    sl = slice(g * EH, (g + 1) * EH)
    wk_pair = _hilo(np.ascontiguousarray(Wk[sl, :].T))
    return {
        "xqt": np.ascontiguousarray(query_n.T).astype(E4M3),
        "xkt": np.ascontiguousarray(kc_n.T).astype(E4M3),
        "xvt": vc_n.T.astype(F16, order="C"),
        "wqt": _hilo(np.ascontiguousarray(Wq[sl, :].T)),
        "wkt0": np.ascontiguousarray(wk_pair[:, :, :EH // 2]),
        "wkt1": np.ascontiguousarray(wk_pair[:, :, EH // 2:]),
        "wvt": Wv[sl, :].T.astype(F16, order="C"),
        "wot": Wo[:, sl].T.astype(F16, order="C"),
        "mbias": np.ascontiguousarray(mb_n.reshape(LK // P, P).T,
                                      dtype=np.float32),
    }


def _shard_inputs(query, keys, values, mask, Wq, Wk, Wv, Wo):
    """Build the 8 per-core input maps.

    Masked-out keys are dropped entirely (they contribute exactly zero to
    both the softmax numerator and denominator), and key/value sequences are
    zero-padded to a common length LK (multiple of 128); the pad positions
    are suppressed through the exp mask-bias.
    """
    N, L, E = query.shape
    HL = 16 // G
    nks = [int(mask[n].sum()) for n in range(N)]
    LK = max(P, -(-max(nks) // P) * P)
    LK = min(LK, L)

    per_batch = []
    for n in range(N):
        if LK == L and nks[n] == L:
            kc, vc = keys[n], values[n]
            mb = np.zeros(L, np.float32)
        else:
            idx = np.flatnonzero(mask[n] != 0)[:LK]
            kc = np.zeros((LK, E), np.float32)
            vc = np.zeros((LK, E), np.float32)
            kc[:idx.size] = keys[n][idx]
            vc[:idx.size] = values[n][idx]
            mb = np.full(LK, MASK_BIAS, np.float32)
            mb[:idx.size] = 0.0
        per_batch.append((kc, vc, mb))

    in_maps = []
    for c in range(NCORES):
        n, g = divmod(c, G)
        kc, vc, mb = per_batch[n]
        in_maps.append(_core_inputs(
            query[n], kc, vc, mb, Wq, Wk, Wv, Wo, g, HL, LK))
    return in_maps, L, E, HL, LK


def kernel(query, keys, values, mask, Wq, Wk, Wv, Wo, bo):
    from concourse.bass_utils import run_bass_kernel_spmd

    query = np.asarray(query, dtype=np.float32)
    keys = np.asarray(keys, dtype=np.float32)
    values = np.asarray(values, dtype=np.float32)
    mask = np.asarray(mask)
    Wq = np.asarray(Wq, dtype=np.float32)
    Wk = np.asarray(Wk, dtype=np.float32)
    Wv = np.asarray(Wv, dtype=np.float32)
    Wo = np.asarray(Wo, dtype=np.float32)
    bo = np.asarray(bo, dtype=np.float32)

    in_maps, L, E, HL, LK = _shard_inputs(
        query, keys, values, mask, Wq, Wk, Wv, Wo)
    nc = _get_nc(L, E, HL, LK)

    res = run_bass_kernel_spmd(nc, in_maps, core_ids=list(range(NCORES)))

    N = query.shape[0]
    out = np.empty((N, L, E), np.float32)
    for n in range(N):
        acc = res.results[G * n]["out"].astype(np.float32)
        for g in range(1, G):
            acc += res.results[G * n + g]["out"].astype(np.float32)
        out[n] = acc + bo[None, :]
    return out
